# revision 19
# baseline (speedup 1.0000x reference)
"""2D valid cross-correlation (16x16 kernel, single channel) on 8 TRN2 cores.

Strategy: shard output rows across the 8 cores; each core's input slice
carries a 15-row halo (built host-side from the full image, so no on-device
halo exchange is needed). On each core the conv is computed on the tensor
engine as banded-Toeplitz matmuls contracting over image rows:

    y[i0+m, j0+n] = sum_b sum_k T_b[k, m] * x[i0+k, j0+n+b]

where T_b[k, m] = w[k-m, b] for 0 <= k-m < 16 (built host-side from the
runtime weights). For each output tile of [113 rows x 512 cols], 16
matmuls (one per kernel column b) accumulate into one PSUM bank; the shift
by b is a free-dim offset on the rhs access pattern, so the same SBUF
image tile serves all 16 taps. float32r runs the PE at 1 cycle/row
(~3.2x faster than fp32 matmul) at ~1e-3 scale-relative accuracy.

Output rows of a block are evicted into one wide SBUF tile and stored as a
single contiguous DRAM transfer per block (strided small stores serialize
onto one DMA engine at ~22 GB/s; contiguous transfers fan out across SDMA
engines). The scratch [block, 113*4096] layout is re-stitched on the host.
"""

from contextlib import ExitStack

import numpy as np

import concourse.tile as tile
from concourse import bacc, mybir
from concourse import bass_utils

H = W = 4096
KH = KW = 16
OH = OW = H - KH + 1          # 4081
NCORES = 8
RPC = 511                     # output rows per core (8*511 = 4088 >= 4081)
M_BLK = 113                   # output rows per lhsT block (128 - (KH-1))
T_STRIDE = 114                # per-tap stride in the Toeplitz tile (8B-aligned)
N_TILE = 512                  # output cols per PSUM tile (one fp32 bank)
N_BLOCKS = (RPC + M_BLK - 1) // M_BLK  # 5 (last block: 59 valid rows)
# All blocks run full K=128/M=113 shapes: partial fp32r matmuls (K=74/M=59)
# were measured at half speed (427ns vs 234ns at N=512). The input is
# zero-padded host-side; garbage output rows are dropped in the stitch.
IN_ROWS = N_BLOCKS * M_BLK + KH - 1  # 580 input rows per core
# fp32r matmuls need even free counts; last tile is 498 wide with a
# 1-column overlap (col 3583 written twice with the identical value).
W_TILES = [(j, 512) for j in range(0, 3584, 512)] + [(3583, 498)]

F32 = mybir.dt.float32
F32R = mybir.dt.float32r

_cache = {}


def _build(mm_dt):
    nc = bacc.Bacc("TRN2", target_bir_lowering=False, debug=False)
    x_d = nc.dram_tensor("x", [IN_ROWS, W], mm_dt, kind="ExternalInput")
    t_d = nc.dram_tensor("tw", [128, KW * T_STRIDE], mm_dt, kind="ExternalInput")
    b_d = nc.dram_tensor("bias", [128, 1], F32, kind="ExternalInput")
    y_d = nc.dram_tensor(
        "y", [N_BLOCKS * len(W_TILES), M_BLK * N_TILE], F32, kind="ExternalOutput"
    )

    with tile.TileContext(nc) as tc, ExitStack() as ctx:
        const_pool = ctx.enter_context(tc.tile_pool(name="const", bufs=1))
        x_pool = ctx.enter_context(tc.tile_pool(name="xblk", bufs=2))
        ev_pool = ctx.enter_context(tc.tile_pool(name="evict", bufs=6))
        ps_pool = ctx.enter_context(tc.tile_pool(name="acc", bufs=8, space="PSUM"))

        # T/bias on the scalar HWDGE ring so they transfer in parallel with
        # block 0's image rows on the sync ring
        t_t = const_pool.tile([128, KW * T_STRIDE], mm_dt)
        nc.scalar.dma_start(t_t[:], t_d[:])
        b_t = const_pool.tile([128, 1], F32)
        nc.scalar.dma_start(b_t[:], b_d[:])

        for t in range(N_BLOCKS):
            m = M_BLK
            k = m + KH - 1  # 128
            x_t = x_pool.tile([128, W], mm_dt, tag="xblk")
            # 128KB descriptors spread each load across SDMA engines; block 0
            # is latency-critical (gates the first matmul), so split it into
            # two instructions to engage two engine groups in parallel
            r_split = [0, 64, k] if t == 0 else [0, k]
            for ra, rb in zip(r_split, r_split[1:]):
                nc.sync.dma_start(
                    x_t[ra:rb, :], x_d[t * M_BLK + ra : t * M_BLK + rb, :],
                    max_dma_last_dim=32768,
                )
            for ji, (j0, nj) in enumerate(W_TILES):
                acc = ps_pool.tile([M_BLK, N_TILE], F32, tag="acc")
                for b in range(KW):
                    nc.tensor.matmul(
                        acc[:m, :nj],
                        t_t[:k, b * T_STRIDE : b * T_STRIDE + m],
                        x_t[:k, j0 + b : j0 + b + nj],
                        start=(b == 0),
                        stop=(b == KW - 1),
                    )
                o_t = ev_pool.tile([M_BLK, N_TILE], F32, tag="out")
                nc.scalar.activation(
                    o_t[:m, :nj],
                    acc[:m, :nj],
                    mybir.ActivationFunctionType.Identity,
                    bias=b_t[:m, 0:1],
                )
                # contiguous per-tile store; SWDGE round-robins one engine per
                # instruction, so stores pipeline with compute. The final tile
                # is split in 4 so its transfer doesn't serialize the tail.
                row = t * len(W_TILES) + ji
                last = t == N_BLOCKS - 1 and ji == len(W_TILES) - 1
                n_chunk = 4 if last else 1
                step = (m + n_chunk - 1) // n_chunk
                for r0 in range(0, m, step):
                    r1 = min(r0 + step, m)
                    nc.gpsimd.dma_start(
                        y_d[row : row + 1, r0 * N_TILE : r1 * N_TILE],
                        o_t[r0:r1, :],
                    )
    nc.compile()
    return nc


def _toeplitz(weight):
    t = np.zeros((128, KW * T_STRIDE), dtype=np.float32)
    idx = np.arange(M_BLK)
    for b in range(KW):
        for a in range(KH):
            t[idx + a, b * T_STRIDE + idx] = weight[a, b]
    return t


def _make_in_maps(x, weight, bias):
    x = np.ascontiguousarray(np.asarray(x, dtype=np.float32))
    weight = np.asarray(weight, dtype=np.float32)
    bias = np.asarray(bias, dtype=np.float32)

    tw = _toeplitz(weight)
    bias_full = np.full((128, 1), float(bias[0]), dtype=np.float32)

    pad_rows = (NCORES - 1) * RPC + IN_ROWS - H  # rows past the image end
    x_pad = np.concatenate(
        [x, np.zeros((pad_rows, W), dtype=np.float32)], axis=0
    )
    in_maps = []
    for c in range(NCORES):
        r0 = c * RPC
        in_maps.append(
            {
                "x": np.ascontiguousarray(x_pad[r0 : r0 + IN_ROWS]),
                "tw": tw,
                "bias": bias_full,
            }
        )
    return in_maps


def _stitch(results):
    y = np.empty((NCORES * RPC, OW), dtype=np.float32)
    for c, r in enumerate(results):
        s = r["y"].reshape(N_BLOCKS, len(W_TILES), M_BLK, N_TILE)
        r0 = c * RPC
        for t in range(N_BLOCKS):
            m = min(M_BLK, RPC - t * M_BLK)
            rows = slice(r0 + t * M_BLK, r0 + t * M_BLK + m)
            for ji, (j0, nj) in enumerate(W_TILES):
                y[rows, j0 : j0 + nj] = s[t, ji, :m, :nj]
    return y[:OH]


def run_sharded(x, weight, bias, mm_dt=F32R, trace=False, trace_cores=None):
    """Returns (y_full, BassKernelResults)."""
    key = str(mm_dt)
    if key not in _cache:
        _cache[key] = _build(mm_dt)
    nc = _cache[key]
    in_maps = _make_in_maps(x, weight, bias)
    res = bass_utils.run_bass_kernel_spmd(
        nc, in_maps, list(range(NCORES)), trace=trace, trace_cores=trace_cores
    )
    return _stitch(res.results), res


def kernel(x, weight, bias):
    y, _ = run_sharded(x, weight, bias)
    return y


# revision 20
# speedup vs baseline: 1.0344x; 1.0344x over previous
"""2D valid cross-correlation (16x16 kernel, single channel) on 8 TRN2 cores.

Strategy: shard output rows across the 8 cores; each core's input slice
carries a 15-row halo (built host-side from the full image, so no on-device
halo exchange is needed). On each core the conv is computed on the tensor
engine as banded-Toeplitz matmuls contracting over image rows:

    y[i0+m, j0+n] = sum_b sum_k T_b[k, m] * x[i0+k, j0+n+b]

where T_b[k, m] = w[k-m, b] for 0 <= k-m < 16 (built host-side from the
runtime weights). For each output tile of [113 rows x 512 cols], 16
matmuls (one per kernel column b) accumulate into one PSUM bank; the shift
by b is a free-dim offset on the rhs access pattern, so the same SBUF
image tile serves all 16 taps. float32r runs the PE at 1 cycle/row
(~3.2x faster than fp32 matmul) at ~1e-4 scale-relative accuracy.

DMA notes (measured): small strided stores serialize onto one SDMA engine
at ~22 GB/s, while contiguous transfers split into 256KB descriptors that
fan out across engines. So each block's rows are evicted into one wide
SBUF tile and stored as 8 contiguous row-chunks via gpsimd (SWDGE
round-robins one engine per instruction) into a block-major scratch that
the host re-stitches. Partial-shape fp32r matmuls (K=74/M=59) run at half
speed, so all blocks use full K=128/M=113 with zero-padded input rows.
"""

from contextlib import ExitStack

import numpy as np

import concourse.tile as tile
from concourse import bacc, mybir
from concourse import bass_utils

H = W = 4096
KH = KW = 16
OH = OW = H - KH + 1          # 4081
NCORES = 8
RPC = 511                     # output rows per core (8*511 = 4088 >= 4081)
M_BLK = 113                   # output rows per lhsT block (128 - (KH-1))
T_STRIDE = 114                # per-tap stride in the Toeplitz tile (8B-aligned)
N_TILE = 512                  # output cols per PSUM tile (one fp32 bank)
N_BLOCKS = (RPC + M_BLK - 1) // M_BLK  # 5 (last block: 59 valid rows)
IN_ROWS = N_BLOCKS * M_BLK + KH - 1    # 580 input rows per core
# fp32r matmuls need even free counts; last tile is 498 wide with a
# 1-column overlap (col 3583 written twice with the identical value).
W_TILES = [(j, 512) for j in range(0, 3584, 512)] + [(3583, 498)]

F32 = mybir.dt.float32
F32R = mybir.dt.float32r

_cache = {}


def _build(mm_dt):
    nc = bacc.Bacc("TRN2", target_bir_lowering=False, debug=False)
    x_d = nc.dram_tensor("x", [IN_ROWS, W], mm_dt, kind="ExternalInput")
    t_d = nc.dram_tensor("tw", [128, KW * T_STRIDE], mm_dt, kind="ExternalInput")
    b_d = nc.dram_tensor("bias", [128, 1], F32, kind="ExternalInput")
    y_d = nc.dram_tensor("y", [N_BLOCKS, M_BLK * W], F32, kind="ExternalOutput")

    with tile.TileContext(nc) as tc, ExitStack() as ctx:
        const_pool = ctx.enter_context(tc.tile_pool(name="const", bufs=1))
        x_pool = ctx.enter_context(tc.tile_pool(name="xblk", bufs=2))
        ev_pool = ctx.enter_context(tc.tile_pool(name="evict", bufs=3))
        ps_pool = ctx.enter_context(tc.tile_pool(name="acc", bufs=8, space="PSUM"))

        # T/bias on the scalar HWDGE ring so they transfer in parallel with
        # block 0's image rows on the sync ring
        t_t = const_pool.tile([128, KW * T_STRIDE], mm_dt)
        nc.scalar.dma_start(t_t[:], t_d[:])
        b_t = const_pool.tile([128, 1], F32)
        nc.scalar.dma_start(b_t[:], b_d[:])

        for t in range(N_BLOCKS):
            m = M_BLK
            k = m + KH - 1  # 128
            x_t = x_pool.tile([128, W], mm_dt, tag="xblk")
            nc.sync.dma_start(x_t[:k, :], x_d[t * M_BLK : t * M_BLK + k, :])
            o_t = ev_pool.tile([M_BLK, W], F32, tag="out")
            for j0, nj in W_TILES:
                acc = ps_pool.tile([M_BLK, N_TILE], F32, tag="acc")
                for b in range(KW):
                    nc.tensor.matmul(
                        acc[:m, :nj],
                        t_t[:k, b * T_STRIDE : b * T_STRIDE + m],
                        x_t[:k, j0 + b : j0 + b + nj],
                        start=(b == 0),
                        stop=(b == KW - 1),
                    )
                nc.scalar.activation(
                    o_t[:m, j0 : j0 + nj],
                    acc[:m, :nj],
                    mybir.ActivationFunctionType.Identity,
                    bias=b_t[:m, 0:1],
                )
            # store in row-chunks, each contiguous in DRAM; SWDGE round-robins
            # one DMA engine per instruction, so 8 chunks engage 8 engines
            n_chunk = 8
            step = (m + n_chunk - 1) // n_chunk
            for r0 in range(0, m, step):
                r1 = min(r0 + step, m)
                nc.gpsimd.dma_start(
                    y_d[t : t + 1, r0 * W : r1 * W], o_t[r0:r1, :]
                )
    nc.compile()
    return nc


def _toeplitz(weight):
    t = np.zeros((128, KW * T_STRIDE), dtype=np.float32)
    idx = np.arange(M_BLK)
    for b in range(KW):
        for a in range(KH):
            t[idx + a, b * T_STRIDE + idx] = weight[a, b]
    return t


def _make_in_maps(x, weight, bias):
    x = np.ascontiguousarray(np.asarray(x, dtype=np.float32))
    weight = np.asarray(weight, dtype=np.float32)
    bias = np.asarray(bias, dtype=np.float32)

    tw = _toeplitz(weight)
    bias_full = np.full((128, 1), float(bias[0]), dtype=np.float32)

    pad_rows = (NCORES - 1) * RPC + IN_ROWS - H  # rows past the image end
    x_pad = np.concatenate(
        [x, np.zeros((pad_rows, W), dtype=np.float32)], axis=0
    )
    in_maps = []
    for c in range(NCORES):
        r0 = c * RPC
        in_maps.append(
            {
                "x": np.ascontiguousarray(x_pad[r0 : r0 + IN_ROWS]),
                "tw": tw,
                "bias": bias_full,
            }
        )
    return in_maps


def _stitch(results):
    y = np.empty((NCORES * RPC, OW), dtype=np.float32)
    for c, r in enumerate(results):
        s = r["y"].reshape(N_BLOCKS, M_BLK, W)
        r0 = c * RPC
        for t in range(N_BLOCKS):
            m = min(M_BLK, RPC - t * M_BLK)
            y[r0 + t * M_BLK : r0 + t * M_BLK + m] = s[t, :m, :OW]
    return y[:OH]


def run_sharded(x, weight, bias, mm_dt=F32R, trace=False, trace_cores=None):
    """Returns (y_full, BassKernelResults)."""
    key = str(mm_dt)
    if key not in _cache:
        _cache[key] = _build(mm_dt)
    nc = _cache[key]
    in_maps = _make_in_maps(x, weight, bias)
    res = bass_utils.run_bass_kernel_spmd(
        nc, in_maps, list(range(NCORES)), trace=trace, trace_cores=trace_cores
    )
    return _stitch(res.results), res


def kernel(x, weight, bias):
    y, _ = run_sharded(x, weight, bias)
    return y


# revision 22
# speedup vs baseline: 1.0482x; 1.0133x over previous
"""2D valid cross-correlation (16x16 kernel, single channel) on 8 TRN2 cores.

Strategy: shard output rows across the 8 cores; each core's input slice
carries a 15-row halo (built host-side from the full image, so no on-device
halo exchange is needed). On each core the conv is computed on the tensor
engine as banded-Toeplitz matmuls contracting over image rows:

    y[i0+m, j0+n] = sum_b sum_k T_b[k, m] * x[i0+k, j0+n+b]

where T_b[k, m] = w[k-m, b] for 0 <= k-m < 16 (built host-side from the
runtime weights). For each output tile of [113 rows x 512 cols], 16
matmuls (one per kernel column b) accumulate into one PSUM bank; the shift
by b is a free-dim offset on the rhs access pattern, so the same SBUF
image tile serves all 16 taps. float32r runs the PE at 1 cycle/row
(~3.2x faster than fp32 matmul) at ~1e-4 scale-relative accuracy.

DMA notes (measured): small strided stores serialize onto one SDMA engine
at ~22 GB/s, while contiguous transfers split into 256KB descriptors that
fan out across engines. So each block's rows are evicted into one wide
SBUF tile and stored as 8 contiguous row-chunks via gpsimd (SWDGE
round-robins one engine per instruction) into a block-major scratch that
the host re-stitches. Partial-shape fp32r matmuls (K=74/M=59) run at half
speed, so all blocks use full K=128/M=113 with zero-padded input rows.
"""

from contextlib import ExitStack

import numpy as np

import concourse.tile as tile
from concourse import bacc, mybir
from concourse import bass_utils

H = W = 4096
KH = KW = 16
OH = OW = H - KH + 1          # 4081
NCORES = 8
RPC = 511                     # output rows per core (8*511 = 4088 >= 4081)
M_BLK = 113                   # output rows per lhsT block (128 - (KH-1))
T_STRIDE = 114                # per-tap stride in the Toeplitz tile (8B-aligned)
N_TILE = 512                  # output cols per PSUM tile (one fp32 bank)
N_BLOCKS = (RPC + M_BLK - 1) // M_BLK  # 5 (last block: 59 valid rows)
IN_ROWS = N_BLOCKS * M_BLK + KH - 1    # 580 input rows per core
# fp32r matmuls need even free counts; last tile is 498 wide with a
# 1-column overlap (col 3583 written twice with the identical value).
W_TILES = [(j, 512) for j in range(0, 3584, 512)] + [(3583, 498)]

F32 = mybir.dt.float32
F32R = mybir.dt.float32r

_cache = {}


def _build(mm_dt):
    nc = bacc.Bacc("TRN2", target_bir_lowering=False, debug=False)
    x_d = nc.dram_tensor("x", [IN_ROWS, W], mm_dt, kind="ExternalInput")
    t_d = nc.dram_tensor("tw", [128, KW * T_STRIDE], mm_dt, kind="ExternalInput")
    b_d = nc.dram_tensor("bias", [128, 1], F32, kind="ExternalInput")
    y_d = nc.dram_tensor("y", [N_BLOCKS, M_BLK * W], F32, kind="ExternalOutput")

    with tile.TileContext(nc) as tc, ExitStack() as ctx:
        const_pool = ctx.enter_context(tc.tile_pool(name="const", bufs=1))
        x_pool = ctx.enter_context(tc.tile_pool(name="xblk", bufs=2))
        ev_pool = ctx.enter_context(tc.tile_pool(name="evict", bufs=3))
        ps_pool = ctx.enter_context(tc.tile_pool(name="acc", bufs=8, space="PSUM"))

        # T/bias on the scalar HWDGE ring so they transfer in parallel with
        # block 0's image rows on the sync ring
        t_t = const_pool.tile([128, KW * T_STRIDE], mm_dt)
        nc.scalar.dma_start(t_t[:], t_d[:])
        b_t = const_pool.tile([128, 1], F32)
        nc.scalar.dma_start(b_t[:], b_d[:])

        last_t = N_BLOCKS - 1
        m_last = RPC - last_t * M_BLK  # 59 valid rows in the last block
        for t in range(N_BLOCKS):
            m = M_BLK
            k = m + KH - 1  # 128
            x_t = x_pool.tile([128, W], mm_dt, tag="xblk")
            nc.sync.dma_start(x_t[:k, :], x_d[t * M_BLK : t * M_BLK + k, :])
            if t < last_t:
                o_t = ev_pool.tile([M_BLK, W], F32, tag="out")
            for ji, (j0, nj) in enumerate(W_TILES):
                acc = ps_pool.tile([M_BLK, N_TILE], F32, tag="acc")
                for b in range(KW):
                    nc.tensor.matmul(
                        acc[:m, :nj],
                        t_t[:k, b * T_STRIDE : b * T_STRIDE + m],
                        x_t[:k, j0 + b : j0 + b + nj],
                        start=(b == 0),
                        stop=(b == KW - 1),
                    )
                if t < last_t:
                    nc.scalar.activation(
                        o_t[:m, j0 : j0 + nj],
                        acc[:m, :nj],
                        mybir.ActivationFunctionType.Identity,
                        bias=b_t[:m, 0:1],
                    )
                else:
                    # last block: only 59 rows are valid; evict + store each
                    # width tile immediately (tile-major packing in y_d[t])
                    # so the tail after the final matmul is one small store
                    o2 = ev_pool.tile([M_BLK, N_TILE], F32, tag="out2")
                    nc.scalar.activation(
                        o2[:m_last, :nj],
                        acc[:m_last, :nj],
                        mybir.ActivationFunctionType.Identity,
                        bias=b_t[:m_last, 0:1],
                    )
                    off = ji * m_last * N_TILE
                    nc.gpsimd.dma_start(
                        y_d[t : t + 1, off : off + m_last * N_TILE],
                        o2[:m_last, :],
                    )
            if t < last_t:
                # store in row-chunks, each contiguous in DRAM; SWDGE
                # round-robins one engine per instruction -> 8 engines
                step = (m + 7) // 8
                for r0 in range(0, m, step):
                    r1 = min(r0 + step, m)
                    nc.gpsimd.dma_start(
                        y_d[t : t + 1, r0 * W : r1 * W], o_t[r0:r1, :]
                    )
    nc.compile()
    return nc


def _toeplitz(weight):
    t = np.zeros((128, KW * T_STRIDE), dtype=np.float32)
    idx = np.arange(M_BLK)
    for b in range(KW):
        for a in range(KH):
            t[idx + a, b * T_STRIDE + idx] = weight[a, b]
    return t


def _make_in_maps(x, weight, bias):
    x = np.ascontiguousarray(np.asarray(x, dtype=np.float32))
    weight = np.asarray(weight, dtype=np.float32)
    bias = np.asarray(bias, dtype=np.float32)

    tw = _toeplitz(weight)
    bias_full = np.full((128, 1), float(bias[0]), dtype=np.float32)

    pad_rows = (NCORES - 1) * RPC + IN_ROWS - H  # rows past the image end
    x_pad = np.concatenate(
        [x, np.zeros((pad_rows, W), dtype=np.float32)], axis=0
    )
    in_maps = []
    for c in range(NCORES):
        r0 = c * RPC
        in_maps.append(
            {
                "x": np.ascontiguousarray(x_pad[r0 : r0 + IN_ROWS]),
                "tw": tw,
                "bias": bias_full,
            }
        )
    return in_maps


def _stitch(results):
    last_t = N_BLOCKS - 1
    m_last = RPC - last_t * M_BLK
    y = np.empty((NCORES * RPC, OW), dtype=np.float32)
    for c, r in enumerate(results):
        raw = r["y"]
        s = raw[:last_t].reshape(last_t, M_BLK, W)
        r0 = c * RPC
        for t in range(last_t):
            y[r0 + t * M_BLK : r0 + (t + 1) * M_BLK] = s[t, :, :OW]
        s4 = raw[last_t, : len(W_TILES) * m_last * N_TILE].reshape(
            len(W_TILES), m_last, N_TILE
        )
        rows = slice(r0 + last_t * M_BLK, r0 + last_t * M_BLK + m_last)
        for ji, (j0, nj) in enumerate(W_TILES):
            y[rows, j0 : j0 + nj] = s4[ji, :, :nj]
    return y[:OH]


def run_sharded(x, weight, bias, mm_dt=F32R, trace=False, trace_cores=None):
    """Returns (y_full, BassKernelResults)."""
    key = str(mm_dt)
    if key not in _cache:
        _cache[key] = _build(mm_dt)
    nc = _cache[key]
    in_maps = _make_in_maps(x, weight, bias)
    res = bass_utils.run_bass_kernel_spmd(
        nc, in_maps, list(range(NCORES)), trace=trace, trace_cores=trace_cores
    )
    return _stitch(res.results), res


def kernel(x, weight, bias):
    y, _ = run_sharded(x, weight, bias)
    return y
